# revision 71
# baseline (speedup 1.0000x reference)
"""
Trainium2 Bass kernel for nn_Attention_6150393168649  (v2: transposed scores).

Transformer-XL-style relative-position attention, b=16 t=512 d=256 h=4 hd=64,
MAX_REL=30.  Data-parallel over batch across 8 NeuronCores (2 batches/core);
weights replicated.

v2 key changes vs v1 (which computed attn[t,s], PE-transposed it, and paid
~12.5us of DVE PSUM->SBUF copies):
  - Scores are computed TRANSPOSED from the start: pssT[s,t] via
    matmul(lhsT=kT chunk, rhs=qTu).  The context matmul consumes the
    exp'd scoresT directly from SBUF -- no attn transposes, no copies.
  - The banded positional multiplier strips are read TRANSPOSED from the
    same t-major skew buffer via dma_start_transpose (16x128 xbar tiles):
    in_[w, j] walks rows t=t_base+w at stride ROWW-1 and 128 contiguous
    key columns; out is [s-chunk 128, t-window 192].  Left-pad 1.0 covers
    t>s+30 cells, right-pad w[t] covers t<=s-30 cells inside the window.
  - Off-window cells t < S0-30 all need m = exp(mtab[t,60]/8) (whole
    columns): folded ADDITIVELY into the score matmul as a rank-1 K=1
    accumulation (ones x arow) before the exp.
  - All matmuls run bf16 lhsT/rhs (1 cycle/row at any N in the PE cost
    model); fp32 PSUM accumulation throughout.
  - Softmax denominators come from per-(head,chunk) ones-column matmuls
    into a shared [8,512] PSUM tile; normalization is pair-packed:
    one reciprocal [2,512], a [2,128]-selector matmul broadcasts both
    heads' 1/den onto 128 partitions, one full-partition multiply.
  - Skew-buffer rows are written in ONE DMA per head ([128,4,317] rows:
    ones-pad | band | w-pad built in a single SBUF tile; gpsimd fills the
    w replication from the ones region).
"""

import math
import sys

import numpy as np

sys.path.insert(0, "/opt/trn_rl_repo")

import ml_dtypes  # noqa: E402

import concourse.bass as bass  # noqa: E402
import concourse.mybir as mybir  # noqa: E402
import concourse.tile as tile  # noqa: E402
from concourse import bacc as _bacc  # noqa: E402
from concourse.bass_utils import run_bass_kernel_spmd  # noqa: E402

# Problem constants (hardcoded per instructions)
B = 16
T = 512
D = 256
H = 4
HD = 64
MAX_REL = 30
NR = 2 * MAX_REL + 1  # 61
NCORES = 8
BPC = B // NCORES  # batches per core
N = BPC * T  # local tokens per core (1024)
P = 128

# skew buffer geometry: row = [ones-pad(128) | band(61) | w-pad(128) | 1]
PAD = 128
ROWW = PAD + NR + PAD + 1  # 318
NSKEW = 8  # one skew buffer per (batch, head)

# strip windows: chunk S0=128*s4 applies the positional multiplier to
# t in [t_lo, t_hi); t < t_lo is covered by the additive arow matmul
TLO = (0, 98, 226, 354)
THI = (158, 286, 414, 512)

FP = mybir.dt.float32
BF = mybir.dt.bfloat16

# bf16 consts block column offsets
WQ_OFF = 0
WK_OFF = 512
WV_OFF = 1024
WO_OFF = 1536
WP_OFF = 2048
ENC_OFF = 2560  # [2, 64] (61 used)
ID_OFF = 2688  # [128] identity
SEL_OFF = 2816  # [128]: row0 = 64x1|64x0, row1 = 64x0|64x1
ROWS_OFF = 2944  # [256]: row0 = bv, row32 = bo
SELM_OFF = 3200  # [8, 8, 128] row-selector: selmat[bh, bh, :] = 1
CBFW = 4224

_CACHE = {}


def _enc_table():
    """61 x 256 sinusoidal table over clipped relative distances (pure
    function of (t, d); mirrors reference._rel_pos_encodings rows)."""
    n = NR
    positions = np.arange(n, dtype=np.float32)[:, None]
    div_term = np.exp(
        np.arange(0, D, 2, dtype=np.float32) * (-math.log(10000.0) / D)
    )
    ang = positions * div_term  # [n, d/2]
    enc = np.stack([np.sin(ang), np.cos(ang)], axis=-1).reshape(n, D)
    return enc.astype(np.float32)  # [61, 256]


def _build_nc():
    # Bacc (not raw Bass): its compile() legalizes multi-wait instructions
    # into standalone event-semaphores (the raw ISA has one wait slot).
    nc = _bacc.Bacc(
        "TRN2", target_bir_lowering=False, debug=False, num_devices=NCORES
    )

    x_ext = nc.declare_dram_parameter("x", [N, D], FP, isOutput=False)
    cb_ext = nc.declare_dram_parameter("cbf", [P, CBFW], BF, isOutput=False)
    cf_ext = nc.declare_dram_parameter("cfp", [P, 16], FP, isOutput=False)
    out_ext = nc.declare_dram_parameter("out", [N, D], FP, isOutput=True)

    skew = nc.dram_tensor("skewbuf", [NSKEW, T, ROWW], BF)

    EXP = mybir.ActivationFunctionType.Exp
    SQRT = mybir.ActivationFunctionType.Sqrt
    SQUARE = mybir.ActivationFunctionType.Square
    IDENT = mybir.ActivationFunctionType.Identity
    MUL = mybir.AluOpType.mult
    ADD = mybir.AluOpType.add
    SUB = mybir.AluOpType.subtract
    AXX = mybir.AxisListType.X

    with nc.allow_low_precision(
        reason="bf16 matmul datapath + bf16 attention probabilities"
    ), tile.TileContext(nc) as tc:
        with (
            tc.tile_pool(name="persist", bufs=1) as pers,
            tc.tile_pool(name="work", bufs=4) as work,
            tc.tile_pool(name="xcp", bufs=1) as xcp,
            tc.tile_pool(name="attnp", bufs=18) as attnp,
            tc.tile_pool(name="small", bufs=8) as small,
            tc.tile_pool(name="psA", bufs=2, space="PSUM") as psA,
            tc.tile_pool(name="psB", bufs=2, space="PSUM") as psB,
            tc.tile_pool(name="psC", bufs=2, space="PSUM") as psC,
            tc.tile_pool(name="psP", bufs=1, space="PSUM") as psP,
        ):
            # ---- x loads first (LN is the longest dependency chain) ----
            nj = N // P  # 8
            hj = 2  # tiles per group
            x_all = pers.tile([P, nj, D], FP, tag="x_all")
            xv = x_ext[:].rearrange("(j p) d -> p j d", p=P)
            for g in range(4):
                gs = slice(hj * g, hj * (g + 1))
                nc.sync.dma_start(out=x_all[:, gs, :], in_=xv[:, gs, :])

            # ---------------- constants (two DMAs) ----------------
            cw = pers.tile([P, CBFW], BF, tag="cw")
            nc.sync.dma_start(out=cw, in_=cb_ext[:])
            cf = pers.tile([P, 16], FP, tag="cf")
            nc.sync.dma_start(out=cf, in_=cf_ext[:])

            def wview(off):
                return cw[:, off : off + 512].rearrange("p (c o) -> p c o", c=2)

            wq_sb = wview(WQ_OFF)
            wk_sb = wview(WK_OFF)
            wv_sb = wview(WV_OFF)
            wo_sb = wview(WO_OFF)
            wp_sb = wview(WP_OFF)
            enc_sb = cw[:, ENC_OFF : ENC_OFF + 128].rearrange(
                "p (c r) -> p c r", c=2
            )
            ident_bf = cw[:, ID_OFF : ID_OFF + 128]
            sel_sb = cw[:, SEL_OFF : SEL_OFF + 128]
            sel_sb32 = cw[32:33, SEL_OFF : SEL_OFF + 128]
            rows_sb = cw[:, ROWS_OFF : ROWS_OFF + 256]
            bv_row = rows_sb[0:1, :]
            bo_row = rows_sb[32:33, :]  # partition 32: PE base-partition rule

            vecs_fp = cf[:].rearrange("p (c k) -> p c k", c=2)
            # vec k: 0 gamma, 1 beta, 2 bq, 3 bk, 4 bpos, 5 ub, 6 vb, 7 spare
            gamma_col = vecs_fp[:, :, 0]
            beta_col = vecs_fp[:, :, 1]
            bk_col = vecs_fp[:, :, 3]
            bp_col = vecs_fp[:, :, 4]

            ones_pad = pers.tile([P, PAD], BF, tag="ones_pad")
            nc.vector.memset(ones_pad, 1.0)
            eps_t = pers.tile([P, 1], FP, tag="eps")
            nc.vector.memset(eps_t, 1e-5)
            # dummy op: pulls the Square/Sqrt act-table load (1.3us) off the
            # LayerNorm critical path by triggering it at t~0
            warm_t = pers.tile([1, 1], FP, tag="warm")
            nc.scalar.activation(out=warm_t, in_=eps_t[0:1, :], func=SQUARE)

            bqu_col = pers.tile([P, 2], FP, tag="bqu")
            nc.vector.tensor_tensor(
                bqu_col, vecs_fp[:, :, 2], vecs_fp[:, :, 5], ADD
            )
            bqv_col = pers.tile([P, 2], FP, tag="bqv")
            nc.vector.tensor_tensor(
                bqv_col, vecs_fp[:, :, 2], vecs_fp[:, :, 6], ADD
            )

            # skew row staging tiles: [ones(128) | band(61) | wrep(128) | 3]
            # one per head: the 8 skew pipelines must not serialize on a
            # shared buffer's band-write DMA round-trip
            skrows = []
            for i in range(8):
                sk = pers.tile([P, 4, 320], BF, tag=f"skrow{i}")
                nc.gpsimd.memset(sk[:, :, 0:PAD], 1.0)
                skrows.append(sk)

            # ---------- LayerNorm + transpose (pipelined) ----------
            xhat = xcp.tile([P, nj, D], BF, tag="xc")
            sq = xcp.tile([P, nj, D], FP, tag="sq")
            xT = pers.tile([P, 2, N], BF, tag="xT")
            for g in range(4):
                gs = slice(hj * g, hj * (g + 1))
                s1 = small.tile([P, hj], FP, tag="s1")
                nc.vector.reduce_sum(out=s1, in_=x_all[:, gs, :], axis=AXX)
                negmu = small.tile([P, hj], FP, tag="negmu")
                nc.vector.tensor_scalar_mul(negmu, s1, -1.0 / D)
                var = small.tile([P, hj], FP, tag="var")
                for jj in range(hj):
                    j = hj * g + jj
                    nc.scalar.activation(
                        out=sq[:, j, :],
                        in_=x_all[:, j, :],
                        func=SQUARE,
                        bias=negmu[:, jj : jj + 1],
                        accum_out=var[:, jj : jj + 1],
                    )
                std = small.tile([P, hj], FP, tag="std")
                nc.scalar.activation(
                    out=std, in_=var, func=SQRT, bias=eps_t[:, 0:1],
                    scale=1.0 / D,
                )
                rs = small.tile([P, hj], FP, tag="rs")
                nc.vector.reciprocal(out=rs, in_=std)
                for jj in range(hj):
                    j = hj * g + jj
                    nc.vector.tensor_scalar(
                        out=xhat[:, j, :],
                        in0=x_all[:, j, :],
                        scalar1=negmu[:, jj : jj + 1],
                        scalar2=rs[:, jj : jj + 1],
                        op0=ADD,
                        op1=MUL,
                    )
                # transpose this group's 2 tiles into xT columns
                # xT[:, c, 256g : 256(g+1)] built from 2 transposes per c
                for c in range(2):
                    ps = psB.tile([P, 2, P], BF, tag="psB")
                    for jj in range(hj):
                        j = hj * g + jj
                        nc.tensor.transpose(
                            ps[:, jj, :],
                            xhat[:, j, P * c : P * (c + 1)],
                            ident_bf,
                        )
                    nc.vector.tensor_scalar(
                        out=xT[:, c, 256 * g : 256 * (g + 1)],
                        in0=ps[:].rearrange("p a b -> p (a b)"),
                        scalar1=gamma_col[:, c : c + 1],
                        scalar2=beta_col[:, c : c + 1],
                        op0=MUL,
                        op1=ADD,
                    )

            # ---------------- posT = (enc @ Wpos.T).T + bpos ----------------
            # (before the q/k projections: the skew pipeline needs posTd)
            posT = pers.tile([P, 2, 64], BF, tag="posT")
            for mc in range(2):
                psp = psA.tile([P, 512], FP, tag="psA")
                for kc in range(2):
                    nc.tensor.matmul(
                        psp[:, 0:64],
                        lhsT=wp_sb[:, kc, P * mc : P * (mc + 1)],
                        rhs=enc_sb[:, kc, :],
                        start=(kc == 0),
                        stop=(kc == 1),
                    )
                nc.vector.tensor_scalar_add(
                    posT[:, mc, :], psp[:, 0:64], bp_col[:, mc : mc + 1]
                )
            # delta table: pos_r - pos_0 (per head-dim); mtab ref is r=0
            posTd = pers.tile([P, 2, 64], BF, tag="posTd")
            nc.vector.tensor_tensor(
                posTd,
                posT,
                posT[:, :, 0:1].to_broadcast(posT.shape),
                SUB,
            )
            # zero-padded posTd[:, 60] columns for the arow matmuls (PE
            # outputs must start at partition 0/32/64, so each (b, oc)
            # matmul uses a [128, 8] lhsT with only its two head-columns
            # nonzero; all four accumulate into one [8, 512] PSUM tile).
            pdzall = pers.tile([P, 4, 8], BF, tag="pdzall")
            nc.vector.memset(pdzall, 0.0)
            for b in range(BPC):
                for oc in range(2):
                    for hh in range(2):
                        po = HD * hh
                        bh = 4 * b + 2 * oc + hh
                        nc.vector.tensor_copy(
                            out=pdzall[po : po + HD, 2 * b + oc, bh : bh + 1],
                            in_=posTd[po : po + HD, oc, 60:61],
                        )

            # ---------------- q/k/v projections (g-outer) ----------------
            # g-outer so batch g's qTv/kT quarters finish first and the
            # skew + attention pipelines for batch 0 start ~7us earlier.
            qTu = pers.tile([P, 2, N], BF, tag="qTu")
            qTv = pers.tile([P, 2, N], BF, tag="qTv")
            kT = pers.tile([P, 2, N], BF, tag="kT")
            # v with a ones column per head: the context matmul's 65th output
            # row is then the softmax denominator for free
            v_sb = pers.tile([P, nj, H, HD + 1], BF, tag="v_sb")
            nc.gpsimd.memset(v_sb, 1.0)
            psar = psP.tile([8, 512], FP, tag="psS", name="psar")
            for g in range(2):
                for oc in range(2):
                    psq = psA.tile([P, 512], FP, tag="psA")
                    psk = psB.tile([P, 512], FP, tag="psB")
                    for kc in range(2):
                        nc.tensor.matmul(
                            psq,
                            lhsT=wq_sb[:, kc, P * oc : P * (oc + 1)],
                            rhs=xT[:, kc, 512 * g : 512 * (g + 1)],
                            start=(kc == 0),
                            stop=(kc == 1),
                        )
                        nc.tensor.matmul(
                            psk,
                            lhsT=wk_sb[:, kc, P * oc : P * (oc + 1)],
                            rhs=xT[:, kc, 512 * g : 512 * (g + 1)],
                            start=(kc == 0),
                            stop=(kc == 1),
                        )
                    sl = (slice(None), oc, slice(512 * g, 512 * (g + 1)))
                    nc.vector.tensor_scalar_add(
                        qTu[sl], psq, bqu_col[:, oc : oc + 1]
                    )
                    nc.scalar.activation(
                        out=qTv[sl],
                        in_=psq,
                        func=IDENT,
                        bias=bqv_col[:, oc : oc + 1],
                    )
                    nc.scalar.activation(
                        out=kT[sl],
                        in_=psk,
                        func=IDENT,
                        bias=bk_col[:, oc : oc + 1],
                    )
                    # arow[bh, t] = mtab[t, 60] rows for batch b = g
                    i = 2 * g + oc
                    nc.tensor.matmul(
                        psar,
                        lhsT=pdzall[:, i, :],
                        rhs=qTv[:, oc, T * g : T * (g + 1)],
                        start=(i == 0),
                        stop=(i == 3),
                    )
                for j in range(4 * g, 4 * g + 4):
                    psv = psB.tile([P, 512], FP, tag="psB")
                    for kc in range(2):
                        nc.tensor.matmul(
                            psv[:, 0:D],
                            lhsT=xT[:, kc, P * j : P * (j + 1)],
                            rhs=wv_sb[:, kc, :],
                            start=(kc == 0),
                            stop=False,
                        )
                    nc.tensor.matmul(
                        psv[:, 0:D],
                        lhsT=ones_pad[0:1, 0:P],
                        rhs=bv_row,
                        start=False,
                        stop=True,
                    )
                    nc.vector.tensor_copy(
                        out=v_sb[:, j, :, 0:HD],
                        in_=psv[:, 0:D].rearrange("p (h d) -> p h d", h=H),
                    )
            arow_sb = pers.tile([8, 512], BF, tag="arow")
            nc.vector.tensor_copy(out=arow_sb, in_=psar)
            # row-selector matrices: selmat[:, bh, :] is [8, 128] with row bh
            # all-ones -- used as lhsT (base partition 0) to broadcast
            # arow_sb[bh] additively into score PSUM columns.
            selmat = cw[0:8, SELM_OFF : SELM_OFF + 1024].rearrange(
                "p (i o) -> p i o", i=8
            )

            # ------- m-tables + skew buffers + transposed strips -------
            # high_priority: the skew pipeline (psp -> exp -> wvals/wrep ->
            # band write -> strip transposes) gates attention's multiplier
            # strips through a DMA round-trip; schedule it ahead of the
            # attention exps whenever both are ready.
            all_strips = {}
            hp_ctx = tc.high_priority()
            hp_ctx.__enter__()
            for bh in range(8):
                    b, h = bh // 4, bh % 4
                    oc, po = h // 2, HD * (h % 2)
                    tb = T * b
                    sk = skrows[bh]
                    psp = psP.tile([P, 4, 64], FP, tag="psP")
                    for t4 in range(4):
                        nc.tensor.matmul(
                            psp[:, t4, :],
                            lhsT=qTv[
                                po : po + HD, oc,
                                tb + P * t4 : tb + P * (t4 + 1),
                            ],
                            rhs=posTd[po : po + HD, oc, :],
                            start=True,
                            stop=True,
                        )
                    nc.scalar.activation(
                        out=sk[:, :, PAD : PAD + NR],
                        in_=psp[:, :, 0:NR],
                        func=EXP,
                        scale=0.125,
                    )
                    wvals = small.tile([P, 4], FP, tag="wvals")
                    nc.gpsimd.tensor_copy(
                        out=wvals, in_=sk[:, :, PAD + NR - 1]
                    )
                    for t4 in range(4):
                        nc.gpsimd.tensor_scalar_mul(
                            sk[:, t4, PAD + NR : PAD + NR + PAD],
                            sk[:, t4, 0:PAD],
                            wvals[:, t4 : t4 + 1],
                        )
                    nc.sync.dma_start(
                        out=bass.AP(
                            tensor=skew[bh].tensor,
                            offset=skew[bh].offset,
                            ap=[[ROWW, P], [ROWW * P, 4], [1, ROWW - 1]],
                        ),
                        in_=sk[:, :, 0 : ROWW - 1],
                    )
            # sheared window read-back, one DMA per head: O2[p, t4, c] =
            # skewrow[t = 128*t4 + p, col = c - p], i.e. column c indexes
            # key s = t4*128 - 158 + c along the shifted diagonal.  The
            # aligned 128-col blocks [158:286], [286:414], [30:158] PE-
            # transpose into [s-chunk, t-block] strip pieces for chunks
            # t4, t4+1 and t4-1 respectively (assembled in PSUM later).
            o2s = []
            for bh in range(8):
                o2 = pers.tile([P, 4, 416], BF, tag=f"o2_{bh}")
                nc.sync.dma_start(
                    out=o2,
                    in_=bass.AP(
                        tensor=skew[bh].tensor,
                        offset=skew[bh].offset,
                        ap=[[ROWW - 1, P], [ROWW * P, 4], [1, 416]],
                    ),
                )
                o2s.append(o2)
            hp_ctx.__exit__(None, None, None)

            # ---------------- attention ----------------
            # Software-pipelined: per pair, emit all score matmuls first
            # (exp/mult chase on Act/DVE), then ctx matmuls, then den
            # matmuls; the pair's normalization is deferred until the NEXT
            # pair's mults are queued so DVE's in-order stream never blocks
            # the following pair's softmax path.  Output projections are
            # emitted once both of a batch's pairs are normalized.
            ctxT = pers.tile([P, 2, N], BF, tag="ctxT")

            def emit_outproj(b):
                for j in range(4 * b, 4 * b + 4):
                    pso = psB.tile([P, 512], FP, tag="psB", name=f"pso{j}")
                    for kc in range(2):
                        nc.tensor.matmul(
                            pso[:, 0:D],
                            lhsT=ctxT[:, kc, P * j : P * (j + 1)],
                            rhs=wo_sb[:, kc, :],
                            start=(kc == 0),
                            stop=False,
                        )
                    nc.tensor.matmul(
                        pso[:, 0:D],
                        lhsT=ones_pad[32:33, 0:P],
                        rhs=bo_row,
                        start=False,
                        stop=True,
                    )
                    o_sb = work.tile([P, D], FP, tag="o_sb")
                    nc.vector.tensor_copy(out=o_sb, in_=pso[:, 0:D])
                    nc.sync.dma_start(
                        out=out_ext[P * j : P * (j + 1), :],
                        in_=o_sb,
                    )

            def emit_normalize(st):
                pscs, b, hp = st
                # hh0's denominator is psc0 row 64 (v ones column); hh1's is
                # psc1 row 0 (separate ones matmul).  rdAB packs 1/den at
                # partitions 0 / 32 for the two base-partition-legal
                # broadcast matmuls; every tensor op below is
                # partition-aligned between all its operands.
                rdAB = small.tile([33, 512], BF, tag="rdAB")
                nc.vector.reciprocal(out=rdAB[0:1, :], in_=pscs[0][64:65, :])
                nc.vector.reciprocal(out=rdAB[32:33, :], in_=pscs[1][0:1, :])
                psdb = psB.tile([P, 512], FP, tag="psB", name="psdb")
                nc.tensor.matmul(
                    psdb, lhsT=sel_sb[0:1, :], rhs=rdAB[0:1, :],
                    start=True, stop=False,
                )
                nc.tensor.matmul(
                    psdb, lhsT=sel_sb32, rhs=rdAB[32:33, :],
                    start=False, stop=True,
                )
                denb = work.tile([P, 512], BF, tag="denb")
                nc.vector.tensor_copy(out=denb, in_=psdb)
                nc.vector.tensor_tensor(
                    ctxT[0:HD, hp, T * b : T * (b + 1)],
                    pscs[0][0:HD, :],
                    denb[0:HD, :],
                    MUL,
                )
                nc.vector.tensor_tensor(
                    ctxT[HD:P, hp, T * b : T * (b + 1)],
                    pscs[1][HD:P, :],
                    denb[HD:P, :],
                    MUL,
                )
                if hp == 1:
                    emit_outproj(b)

            pending = None
            for b in range(BPC):
                tb = T * b
                for hp in range(2):
                    oc = hp
                    # psc0: [0:64] ctx hh0, row 64 den hh0 (v ones column)
                    # psc1: row 0 den hh1, [64:128] ctx hh1
                    pscs = [
                        psC.tile([65 + 63 * hh, 512], FP, tag=f"psc{hh}",
                                 bufs=1, name=f"psc{hh}")
                        for hh in range(2)
                    ]
                    ats = {}
                    pss_tiles = {}
                    for s4 in range(4):
                        t_lo, t_hi = TLO[s4], THI[s4]
                        S0 = P * s4
                        for hh in range(2):
                            po = HD * hh
                            bh = 4 * b + 2 * hp + hh
                            pss = psA.tile([P, 512], FP, tag="psA")
                            ksl = kT[po : po + HD, oc, tb + S0 : tb + S0 + P]
                            if t_lo:
                                nc.tensor.matmul(
                                    pss[:, 0:t_lo],
                                    lhsT=ksl,
                                    rhs=qTu[po : po + HD, oc, tb : tb + t_lo],
                                    start=True,
                                    stop=False,
                                )
                                nc.tensor.matmul(
                                    pss[:, 0:t_lo],
                                    lhsT=selmat[:, bh, :],
                                    rhs=arow_sb[:, 0:t_lo],
                                    start=False,
                                    stop=True,
                                )
                            nc.tensor.matmul(
                                pss[:, t_lo:T],
                                lhsT=ksl,
                                rhs=qTu[po : po + HD, oc, tb + t_lo : tb + T],
                                start=True,
                                stop=True,
                            )
                            at = attnp.tile([P, T], BF, tag="attn")
                            nc.scalar.activation(
                                out=at, in_=pss, func=EXP, scale=0.125
                            )
                            # both heads' multiplier strips live in one
                            # [128, 2, 288] PSUM tile per (pair, chunk),
                            # assembled by aligned PE transposes of the
                            # sheared window; col 0 is t = S0-128
                            o2 = o2s[bh]
                            if hh == 0:
                                psS = psP.tile(
                                    [P, 2, 288], BF, tag="psS", name="psS"
                                )
                                pss_tiles[s4] = psS
                            else:
                                psS = pss_tiles[s4]
                            nc.tensor.transpose(
                                psS[:, hh, 128:256],
                                o2[:, s4, 158:286],
                                ident_bf,
                            )
                            if s4 > 0:
                                nc.tensor.transpose(
                                    psS[:, hh, 0:128],
                                    o2[:, s4 - 1, 286:414],
                                    ident_bf,
                                )
                            if s4 < 3:
                                # rows 0:32 of o2's next block: t in
                                # [S0+128, S0+160)
                                nc.tensor.transpose(
                                    psS[:, hh, 256:288],
                                    o2[0:32, s4 + 1, 30:158],
                                    ident_bf[0:32, 0:32],
                                )
                            nc.vector.tensor_tensor(
                                at[:, t_lo:t_hi],
                                at[:, t_lo:t_hi],
                                psS[:, hh, t_lo - S0 + 128 : t_hi - S0 + 128],
                                MUL,
                            )
                            ats[(hh, s4)] = at
                            if hh == 0:
                                nc.tensor.matmul(
                                    pscs[0],
                                    lhsT=v_sb[:, 4 * b + s4, 2 * hp, :],
                                    rhs=at,
                                    start=(s4 == 0),
                                    stop=(s4 == 3),
                                )
                            else:
                                nc.tensor.matmul(
                                    pscs[1][HD:P, :],
                                    lhsT=v_sb[:, 4 * b + s4, 2 * hp + 1,
                                              0:HD],
                                    rhs=at,
                                    start=(s4 == 0),
                                    stop=(s4 == 3),
                                )
                                nc.tensor.matmul(
                                    pscs[1][0:1, :],
                                    lhsT=v_sb[:, 4 * b + s4, 2 * hp + 1,
                                              HD : HD + 1],
                                    rhs=at,
                                    start=(s4 == 0),
                                    stop=(s4 == 3),
                                )
                    if pending is not None:
                        emit_normalize(pending)
                    pending = (pscs, b, hp)
            emit_normalize(pending)
    nc.finalize()
    return nc


def _get_nc():
    if "nc" not in _CACHE:
        _CACHE["nc"] = _build_nc()
    return _CACHE["nc"]


def _make_in_maps(inputs):
    x = np.asarray(inputs["inputs"], dtype=np.float32)  # [16, 512, 256]
    enc = _enc_table()
    bf = ml_dtypes.bfloat16

    def wtile(w):
        # W [o, i] -> W.T [i, o] -> [p, (c o)] with i = c*128 + p
        return (
            np.asarray(w, np.float32)
            .T.reshape(2, P, D)
            .transpose(1, 0, 2)
            .reshape(P, 512)
        )

    def coltile(v):
        return np.asarray(v, np.float32).reshape(2, P).T  # [p, c]

    cbf = np.zeros((P, CBFW), np.float32)
    for off, w in [
        (WQ_OFF, inputs["Wq"]),
        (WK_OFF, inputs["Wk"]),
        (WV_OFF, inputs["Wv"]),
        (WO_OFF, inputs["Wo"]),
        (WP_OFF, inputs["Wpos"]),
    ]:
        cbf[:, off : off + 512] = wtile(w)
    encp = np.zeros((2, P, 64), np.float32)
    encp[:, :, 0:NR] = enc.T.reshape(2, P, NR)
    cbf[:, ENC_OFF : ENC_OFF + 128] = encp.transpose(1, 0, 2).reshape(P, 128)
    cbf[:, ID_OFF : ID_OFF + 128] = np.eye(P, dtype=np.float32)
    cbf[0, SEL_OFF : SEL_OFF + 64] = 1.0
    cbf[32, SEL_OFF + 64 : SEL_OFF + 128] = 1.0
    cbf[0, ROWS_OFF : ROWS_OFF + 256] = np.asarray(inputs["bv"], np.float32)
    cbf[32, ROWS_OFF : ROWS_OFF + 256] = np.asarray(inputs["bo"], np.float32)
    for bh in range(8):
        cbf[bh, SELM_OFF + 128 * bh : SELM_OFF + 128 * (bh + 1)] = 1.0

    vecs = np.zeros((P, 2, 8), np.float32)
    vecs[:, :, 0] = coltile(inputs["ln_gamma"])
    vecs[:, :, 1] = coltile(inputs["ln_beta"])
    vecs[:, :, 2] = coltile(inputs["bq"])
    vecs[:, :, 3] = coltile(inputs["bk"])
    vecs[:, :, 4] = coltile(inputs["bpos"])
    vecs[:, :, 5] = coltile(np.asarray(inputs["u_bias"], np.float32).reshape(D))
    vecs[:, :, 6] = coltile(np.asarray(inputs["v_bias"], np.float32).reshape(D))

    common = {
        "cbf": np.ascontiguousarray(cbf.astype(bf)),
        "cfp": np.ascontiguousarray(vecs.reshape(P, 16)),
    }
    in_maps = []
    for core in range(NCORES):
        m = dict(common)
        m["x"] = np.ascontiguousarray(
            x[BPC * core : BPC * (core + 1)].reshape(N, D)
        )
        in_maps.append(m)
    return in_maps


def run(inputs, trace=False):
    nc = _get_nc()
    in_maps = _make_in_maps(inputs)
    res = run_bass_kernel_spmd(
        nc, in_maps, core_ids=list(range(NCORES)), trace=trace
    )
    outs = [np.asarray(r["out"]) for r in res.results]
    full = np.concatenate(outs, axis=0).reshape(B, T, D).astype(np.float32)
    return full, res


def kernel(**inputs) -> np.ndarray:
    full, _ = run(inputs, trace=False)
    return full


# revision 72
# speedup vs baseline: 1.0873x; 1.0873x over previous
"""
Trainium2 Bass kernel for nn_Attention_6150393168649  (v2: transposed scores).

Transformer-XL-style relative-position attention, b=16 t=512 d=256 h=4 hd=64,
MAX_REL=30.  Data-parallel over batch across 8 NeuronCores (2 batches/core);
weights replicated.

v2 key changes vs v1 (which computed attn[t,s], PE-transposed it, and paid
~12.5us of DVE PSUM->SBUF copies):
  - Scores are computed TRANSPOSED from the start: pssT[s,t] via
    matmul(lhsT=kT chunk, rhs=qTu).  The context matmul consumes the
    exp'd scoresT directly from SBUF -- no attn transposes, no copies.
  - The banded positional multiplier strips are read TRANSPOSED from the
    same t-major skew buffer via dma_start_transpose (16x128 xbar tiles):
    in_[w, j] walks rows t=t_base+w at stride ROWW-1 and 128 contiguous
    key columns; out is [s-chunk 128, t-window 192].  Left-pad 1.0 covers
    t>s+30 cells, right-pad w[t] covers t<=s-30 cells inside the window.
  - Off-window cells t < S0-30 all need m = exp(mtab[t,60]/8) (whole
    columns): folded ADDITIVELY into the score matmul as a rank-1 K=1
    accumulation (ones x arow) before the exp.
  - All matmuls run bf16 lhsT/rhs (1 cycle/row at any N in the PE cost
    model); fp32 PSUM accumulation throughout.
  - Softmax denominators come from per-(head,chunk) ones-column matmuls
    into a shared [8,512] PSUM tile; normalization is pair-packed:
    one reciprocal [2,512], a [2,128]-selector matmul broadcasts both
    heads' 1/den onto 128 partitions, one full-partition multiply.
  - Skew-buffer rows are written in ONE DMA per head ([128,4,317] rows:
    ones-pad | band | w-pad built in a single SBUF tile; gpsimd fills the
    w replication from the ones region).
"""

import math
import sys

import numpy as np

sys.path.insert(0, "/opt/trn_rl_repo")

import ml_dtypes  # noqa: E402

import concourse.bass as bass  # noqa: E402
import concourse.mybir as mybir  # noqa: E402
import concourse.tile as tile  # noqa: E402
from concourse import bacc as _bacc  # noqa: E402
from concourse.bass_utils import run_bass_kernel_spmd  # noqa: E402

# Problem constants (hardcoded per instructions)
B = 16
T = 512
D = 256
H = 4
HD = 64
MAX_REL = 30
NR = 2 * MAX_REL + 1  # 61
NCORES = 8
BPC = B // NCORES  # batches per core
N = BPC * T  # local tokens per core (1024)
P = 128

# skew buffer geometry: row = [ones-pad(128) | band(61) | w-pad(128) | 1]
PAD = 128
ROWW = PAD + NR + PAD + 1  # 318
NSKEW = 8  # one skew buffer per (batch, head)

# strip windows: chunk S0=128*s4 applies the positional multiplier to
# t in [t_lo, t_hi); t < t_lo is covered by the additive arow matmul
TLO = (0, 98, 226, 354)
THI = (158, 286, 414, 512)

FP = mybir.dt.float32
BF = mybir.dt.bfloat16

# bf16 consts block column offsets
WQ_OFF = 0
WK_OFF = 512
WV_OFF = 1024
WO_OFF = 1536
WP_OFF = 2048
ENC_OFF = 2560  # [2, 64] (61 used)
ID_OFF = 2688  # [128] identity
SEL_OFF = 2816  # [128]: row0 = 64x1|64x0, row1 = 64x0|64x1
ROWS_OFF = 2944  # [256]: row0 = bv, row32 = bo
SELM_OFF = 3200  # [8, 8, 128] row-selector: selmat[bh, bh, :] = 1
CBFW = 4224

_CACHE = {}


def _enc_table():
    """61 x 256 sinusoidal table over clipped relative distances (pure
    function of (t, d); mirrors reference._rel_pos_encodings rows)."""
    n = NR
    positions = np.arange(n, dtype=np.float32)[:, None]
    div_term = np.exp(
        np.arange(0, D, 2, dtype=np.float32) * (-math.log(10000.0) / D)
    )
    ang = positions * div_term  # [n, d/2]
    enc = np.stack([np.sin(ang), np.cos(ang)], axis=-1).reshape(n, D)
    return enc.astype(np.float32)  # [61, 256]


def _build_nc():
    # Bacc (not raw Bass): its compile() legalizes multi-wait instructions
    # into standalone event-semaphores (the raw ISA has one wait slot).
    nc = _bacc.Bacc(
        "TRN2", target_bir_lowering=False, debug=False, num_devices=NCORES
    )

    x_ext = nc.declare_dram_parameter("x", [N, D], FP, isOutput=False)
    cb_ext = nc.declare_dram_parameter("cbf", [P, CBFW], BF, isOutput=False)
    cf_ext = nc.declare_dram_parameter("cfp", [P, 16], FP, isOutput=False)
    out_ext = nc.declare_dram_parameter("out", [N, D], FP, isOutput=True)

    skew = nc.dram_tensor("skewbuf", [NSKEW, T, ROWW], BF)

    EXP = mybir.ActivationFunctionType.Exp
    SQRT = mybir.ActivationFunctionType.Sqrt
    SQUARE = mybir.ActivationFunctionType.Square
    IDENT = mybir.ActivationFunctionType.Identity
    MUL = mybir.AluOpType.mult
    ADD = mybir.AluOpType.add
    SUB = mybir.AluOpType.subtract
    AXX = mybir.AxisListType.X

    with nc.allow_low_precision(
        reason="bf16 matmul datapath + bf16 attention probabilities"
    ), tile.TileContext(nc) as tc:
        with (
            tc.tile_pool(name="persist", bufs=1) as pers,
            tc.tile_pool(name="work", bufs=4) as work,
            tc.tile_pool(name="xcp", bufs=1) as xcp,
            tc.tile_pool(name="attnp", bufs=18) as attnp,
            tc.tile_pool(name="small", bufs=8) as small,
            tc.tile_pool(name="psA", bufs=2, space="PSUM") as psA,
            tc.tile_pool(name="psB", bufs=2, space="PSUM") as psB,
            tc.tile_pool(name="psC", bufs=2, space="PSUM") as psC,
            tc.tile_pool(name="psP", bufs=1, space="PSUM") as psP,
        ):
            # ---- x loads first (LN is the longest dependency chain) ----
            nj = N // P  # 8
            hj = 2  # tiles per group
            x_all = pers.tile([P, nj, D], FP, tag="x_all")
            xv = x_ext[:].rearrange("(j p) d -> p j d", p=P)
            for g in range(4):
                gs = slice(hj * g, hj * (g + 1))
                nc.sync.dma_start(out=x_all[:, gs, :], in_=xv[:, gs, :])

            # ---------------- constants (two DMAs) ----------------
            cw = pers.tile([P, CBFW], BF, tag="cw")
            nc.sync.dma_start(out=cw, in_=cb_ext[:])
            cf = pers.tile([P, 16], FP, tag="cf")
            nc.sync.dma_start(out=cf, in_=cf_ext[:])

            def wview(off):
                return cw[:, off : off + 512].rearrange("p (c o) -> p c o", c=2)

            wq_sb = wview(WQ_OFF)
            wk_sb = wview(WK_OFF)
            wv_sb = wview(WV_OFF)
            wo_sb = wview(WO_OFF)
            wp_sb = wview(WP_OFF)
            enc_sb = cw[:, ENC_OFF : ENC_OFF + 128].rearrange(
                "p (c r) -> p c r", c=2
            )
            ident_bf = cw[:, ID_OFF : ID_OFF + 128]
            sel_sb = cw[:, SEL_OFF : SEL_OFF + 128]
            sel_sb32 = cw[32:33, SEL_OFF : SEL_OFF + 128]
            rows_sb = cw[:, ROWS_OFF : ROWS_OFF + 256]
            bv_row = rows_sb[0:1, :]
            bo_row = rows_sb[32:33, :]  # partition 32: PE base-partition rule

            vecs_fp = cf[:].rearrange("p (c k) -> p c k", c=2)
            # vec k: 0 gamma, 1 beta, 2 bq, 3 bk, 4 bpos, 5 ub, 6 vb, 7 spare
            gamma_col = vecs_fp[:, :, 0]
            beta_col = vecs_fp[:, :, 1]
            bk_col = vecs_fp[:, :, 3]
            bp_col = vecs_fp[:, :, 4]

            ones_pad = pers.tile([P, PAD], BF, tag="ones_pad")
            nc.vector.memset(ones_pad, 1.0)
            eps_t = pers.tile([P, 1], FP, tag="eps")
            nc.vector.memset(eps_t, 1e-5)
            # dummy op: pulls the Square/Sqrt act-table load (1.3us) off the
            # LayerNorm critical path by triggering it at t~0
            warm_t = pers.tile([1, 1], FP, tag="warm")
            nc.scalar.activation(out=warm_t, in_=eps_t[0:1, :], func=SQUARE)

            bqu_col = pers.tile([P, 2], FP, tag="bqu")
            nc.vector.tensor_tensor(
                bqu_col, vecs_fp[:, :, 2], vecs_fp[:, :, 5], ADD
            )
            bqv_col = pers.tile([P, 2], FP, tag="bqv")
            nc.vector.tensor_tensor(
                bqv_col, vecs_fp[:, :, 2], vecs_fp[:, :, 6], ADD
            )

            # skew row staging tiles: [ones(128) | band(61) | wrep(128) | 3]
            # one per head: the 8 skew pipelines must not serialize on a
            # shared buffer's band-write DMA round-trip
            skrows = []
            for i in range(8):
                sk = pers.tile([P, 4, 320], BF, tag=f"skrow{i}")
                nc.gpsimd.memset(sk[:, :, 0:PAD], 1.0)
                skrows.append(sk)

            # ---------- LayerNorm + transpose (pipelined) ----------
            xhat = xcp.tile([P, nj, D], BF, tag="xc")
            sq = xcp.tile([P, nj, D], FP, tag="sq")
            xT = pers.tile([P, 2, N], BF, tag="xT")
            for g in range(4):
                gs = slice(hj * g, hj * (g + 1))
                s1 = small.tile([P, hj], FP, tag="s1")
                nc.vector.reduce_sum(out=s1, in_=x_all[:, gs, :], axis=AXX)
                negmu = small.tile([P, hj], FP, tag="negmu")
                nc.vector.tensor_scalar_mul(negmu, s1, -1.0 / D)
                var = small.tile([P, hj], FP, tag="var")
                for jj in range(hj):
                    j = hj * g + jj
                    nc.scalar.activation(
                        out=sq[:, j, :],
                        in_=x_all[:, j, :],
                        func=SQUARE,
                        bias=negmu[:, jj : jj + 1],
                        accum_out=var[:, jj : jj + 1],
                    )
                std = small.tile([P, hj], FP, tag="std")
                nc.scalar.activation(
                    out=std, in_=var, func=SQRT, bias=eps_t[:, 0:1],
                    scale=1.0 / D,
                )
                rs = small.tile([P, hj], FP, tag="rs")
                nc.vector.reciprocal(out=rs, in_=std)
                for jj in range(hj):
                    j = hj * g + jj
                    nc.vector.tensor_scalar(
                        out=xhat[:, j, :],
                        in0=x_all[:, j, :],
                        scalar1=negmu[:, jj : jj + 1],
                        scalar2=rs[:, jj : jj + 1],
                        op0=ADD,
                        op1=MUL,
                    )
                # transpose this group's 2 tiles into xT columns
                # xT[:, c, 256g : 256(g+1)] built from 2 transposes per c
                for c in range(2):
                    ps = psB.tile([P, 2, P], BF, tag="psB")
                    for jj in range(hj):
                        j = hj * g + jj
                        nc.tensor.transpose(
                            ps[:, jj, :],
                            xhat[:, j, P * c : P * (c + 1)],
                            ident_bf,
                        )
                    nc.vector.tensor_scalar(
                        out=xT[:, c, 256 * g : 256 * (g + 1)],
                        in0=ps[:].rearrange("p a b -> p (a b)"),
                        scalar1=gamma_col[:, c : c + 1],
                        scalar2=beta_col[:, c : c + 1],
                        op0=MUL,
                        op1=ADD,
                    )

            # ---------------- posT = (enc @ Wpos.T).T + bpos ----------------
            # (before the q/k projections: the skew pipeline needs posTd)
            posT = pers.tile([P, 2, 64], BF, tag="posT")
            for mc in range(2):
                psp = psA.tile([P, 512], FP, tag="psA")
                for kc in range(2):
                    nc.tensor.matmul(
                        psp[:, 0:64],
                        lhsT=wp_sb[:, kc, P * mc : P * (mc + 1)],
                        rhs=enc_sb[:, kc, :],
                        start=(kc == 0),
                        stop=(kc == 1),
                    )
                nc.vector.tensor_scalar_add(
                    posT[:, mc, :], psp[:, 0:64], bp_col[:, mc : mc + 1]
                )
            # delta table: pos_r - pos_0 (per head-dim); mtab ref is r=0
            posTd = pers.tile([P, 2, 64], BF, tag="posTd")
            nc.vector.tensor_tensor(
                posTd,
                posT,
                posT[:, :, 0:1].to_broadcast(posT.shape),
                SUB,
            )
            # zero-padded posTd[:, 60] columns for the arow matmuls (PE
            # outputs must start at partition 0/32/64, so each (b, oc)
            # matmul uses a [128, 8] lhsT with only its two head-columns
            # nonzero; all four accumulate into one [8, 512] PSUM tile).
            pdzall = pers.tile([P, 4, 8], BF, tag="pdzall")
            nc.vector.memset(pdzall, 0.0)
            for b in range(BPC):
                for oc in range(2):
                    for hh in range(2):
                        po = HD * hh
                        bh = 4 * b + 2 * oc + hh
                        nc.vector.tensor_copy(
                            out=pdzall[po : po + HD, 2 * b + oc, bh : bh + 1],
                            in_=posTd[po : po + HD, oc, 60:61],
                        )

            # ---------------- q/k/v projections (g-outer) ----------------
            # g-outer so batch g's qTv/kT quarters finish first and the
            # skew + attention pipelines for batch 0 start ~7us earlier.
            qTu = pers.tile([P, 2, N], BF, tag="qTu")
            qTv = pers.tile([P, 2, N], BF, tag="qTv")
            kT = pers.tile([P, 2, N], BF, tag="kT")
            # v with a ones column per head: the context matmul's 65th output
            # row is then the softmax denominator for free
            v_sb = pers.tile([P, nj, H, HD + 1], BF, tag="v_sb")
            nc.gpsimd.memset(v_sb, 1.0)
            psar = psP.tile([8, 512], FP, tag="psS", name="psar")
            for g in range(2):
                for oc in range(2):
                    psq = psA.tile([P, 512], FP, tag="psA")
                    psk = psB.tile([P, 512], FP, tag="psB")
                    for kc in range(2):
                        nc.tensor.matmul(
                            psq,
                            lhsT=wq_sb[:, kc, P * oc : P * (oc + 1)],
                            rhs=xT[:, kc, 512 * g : 512 * (g + 1)],
                            start=(kc == 0),
                            stop=(kc == 1),
                        )
                        nc.tensor.matmul(
                            psk,
                            lhsT=wk_sb[:, kc, P * oc : P * (oc + 1)],
                            rhs=xT[:, kc, 512 * g : 512 * (g + 1)],
                            start=(kc == 0),
                            stop=(kc == 1),
                        )
                    sl = (slice(None), oc, slice(512 * g, 512 * (g + 1)))
                    nc.vector.tensor_scalar_add(
                        qTu[sl], psq, bqu_col[:, oc : oc + 1]
                    )
                    nc.scalar.activation(
                        out=qTv[sl],
                        in_=psq,
                        func=IDENT,
                        bias=bqv_col[:, oc : oc + 1],
                    )
                    nc.scalar.activation(
                        out=kT[sl],
                        in_=psk,
                        func=IDENT,
                        bias=bk_col[:, oc : oc + 1],
                    )
                    # arow[bh, t] = mtab[t, 60] rows for batch b = g
                    i = 2 * g + oc
                    nc.tensor.matmul(
                        psar,
                        lhsT=pdzall[:, i, :],
                        rhs=qTv[:, oc, T * g : T * (g + 1)],
                        start=(i == 0),
                        stop=(i == 3),
                    )
                for j in range(4 * g, 4 * g + 4):
                    psv = psB.tile([P, 512], FP, tag="psB")
                    for kc in range(2):
                        nc.tensor.matmul(
                            psv[:, 0:D],
                            lhsT=xT[:, kc, P * j : P * (j + 1)],
                            rhs=wv_sb[:, kc, :],
                            start=(kc == 0),
                            stop=False,
                        )
                    nc.tensor.matmul(
                        psv[:, 0:D],
                        lhsT=ones_pad[0:1, 0:P],
                        rhs=bv_row,
                        start=False,
                        stop=True,
                    )
                    nc.vector.tensor_copy(
                        out=v_sb[:, j, :, 0:HD],
                        in_=psv[:, 0:D].rearrange("p (h d) -> p h d", h=H),
                    )
            arow_sb = pers.tile([8, 512], BF, tag="arow")
            nc.vector.tensor_copy(out=arow_sb, in_=psar)
            # row-selector matrices: selmat[:, bh, :] is [8, 128] with row bh
            # all-ones -- used as lhsT (base partition 0) to broadcast
            # arow_sb[bh] additively into score PSUM columns.
            selmat = cw[0:8, SELM_OFF : SELM_OFF + 1024].rearrange(
                "p (i o) -> p i o", i=8
            )

            # ------- m-tables + skew buffers + transposed strips -------
            # high_priority: the skew pipeline (psp -> exp -> wvals/wrep ->
            # band write -> strip transposes) gates attention's multiplier
            # strips through a DMA round-trip; schedule it ahead of the
            # attention exps whenever both are ready.
            all_strips = {}
            hp_ctx = tc.high_priority()
            hp_ctx.__enter__()
            for bh in range(8):
                    b, h = bh // 4, bh % 4
                    oc, po = h // 2, HD * (h % 2)
                    tb = T * b
                    sk = skrows[bh]
                    psp = psP.tile([P, 4, 64], FP, tag="psP")
                    for t4 in range(4):
                        nc.tensor.matmul(
                            psp[:, t4, :],
                            lhsT=qTv[
                                po : po + HD, oc,
                                tb + P * t4 : tb + P * (t4 + 1),
                            ],
                            rhs=posTd[po : po + HD, oc, :],
                            start=True,
                            stop=True,
                        )
                    nc.scalar.activation(
                        out=sk[:, :, PAD : PAD + NR],
                        in_=psp[:, :, 0:NR],
                        func=EXP,
                        scale=0.125,
                    )
                    wvals = small.tile([P, 4], FP, tag="wvals")
                    nc.gpsimd.tensor_copy(
                        out=wvals, in_=sk[:, :, PAD + NR - 1]
                    )
                    for t4 in range(4):
                        nc.gpsimd.tensor_scalar_mul(
                            sk[:, t4, PAD + NR : PAD + NR + PAD],
                            sk[:, t4, 0:PAD],
                            wvals[:, t4 : t4 + 1],
                        )
                    nc.sync.dma_start(
                        out=bass.AP(
                            tensor=skew[bh].tensor,
                            offset=skew[bh].offset,
                            ap=[[ROWW, P], [ROWW * P, 4], [1, ROWW - 1]],
                        ),
                        in_=sk[:, :, 0 : ROWW - 1],
                    )
            # sheared window read-back, one DMA per head: O2[p, t4, c] =
            # skewrow[t = 128*t4 + p, col = c - p], i.e. column c indexes
            # key s = t4*128 - 158 + c along the shifted diagonal.  The
            # aligned 128-col blocks [158:286], [286:414], [30:158] PE-
            # transpose into [s-chunk, t-block] strip pieces for chunks
            # t4, t4+1 and t4-1 respectively (assembled in PSUM later).
            o2s = []
            for bh in range(8):
                o2 = pers.tile([P, 4, 416], BF, tag=f"o2_{bh}")
                nc.sync.dma_start(
                    out=o2,
                    in_=bass.AP(
                        tensor=skew[bh].tensor,
                        offset=skew[bh].offset,
                        ap=[[ROWW - 1, P], [ROWW * P, 4], [1, 416]],
                    ),
                )
                o2s.append(o2)
            hp_ctx.__exit__(None, None, None)

            # ---------------- attention ----------------
            # Software-pipelined: per pair, emit all score matmuls first
            # (exp/mult chase on Act/DVE), then ctx matmuls, then den
            # matmuls; the pair's normalization is deferred until the NEXT
            # pair's mults are queued so DVE's in-order stream never blocks
            # the following pair's softmax path.  Output projections are
            # emitted once both of a batch's pairs are normalized.
            ctxT = pers.tile([P, 2, N], BF, tag="ctxT")

            def emit_outproj(b):
                for j in range(4 * b, 4 * b + 4):
                    pso = psB.tile([P, 512], FP, tag="psB", name=f"pso{j}")
                    for kc in range(2):
                        nc.tensor.matmul(
                            pso[:, 0:D],
                            lhsT=ctxT[:, kc, P * j : P * (j + 1)],
                            rhs=wo_sb[:, kc, :],
                            start=(kc == 0),
                            stop=False,
                        )
                    nc.tensor.matmul(
                        pso[:, 0:D],
                        lhsT=ones_pad[32:33, 0:P],
                        rhs=bo_row,
                        start=False,
                        stop=True,
                    )
                    o_sb = work.tile([P, D], FP, tag="o_sb")
                    nc.vector.tensor_copy(out=o_sb, in_=pso[:, 0:D])
                    nc.sync.dma_start(
                        out=out_ext[P * j : P * (j + 1), :],
                        in_=o_sb,
                    )

            def emit_normalize(st):
                pscs, b, hp = st
                # hh0's denominator is psc0 row 64 (v ones column); hh1's is
                # psc1 row 0 (separate ones matmul).  rdAB packs 1/den at
                # partitions 0 / 32 for the two base-partition-legal
                # broadcast matmuls; every tensor op below is
                # partition-aligned between all its operands.
                rdAB = small.tile([33, 512], BF, tag="rdAB")
                nc.vector.reciprocal(out=rdAB[0:1, :], in_=pscs[0][64:65, :])
                nc.vector.reciprocal(out=rdAB[32:33, :], in_=pscs[1][0:1, :])
                psdb = psB.tile([P, 512], FP, tag="psB", name="psdb")
                nc.tensor.matmul(
                    psdb, lhsT=sel_sb[0:1, :], rhs=rdAB[0:1, :],
                    start=True, stop=False,
                )
                nc.tensor.matmul(
                    psdb, lhsT=sel_sb32, rhs=rdAB[32:33, :],
                    start=False, stop=True,
                )
                denb = work.tile([P, 512], BF, tag="denb")
                nc.vector.tensor_copy(out=denb, in_=psdb)
                nc.vector.tensor_tensor(
                    ctxT[0:HD, hp, T * b : T * (b + 1)],
                    pscs[0][0:HD, :],
                    denb[0:HD, :],
                    MUL,
                )
                nc.vector.tensor_tensor(
                    ctxT[HD:P, hp, T * b : T * (b + 1)],
                    pscs[1][HD:P, :],
                    denb[HD:P, :],
                    MUL,
                )
                if hp == 1:
                    emit_outproj(b)

            pending = None
            for b in range(BPC):
                tb = T * b
                for hp in range(2):
                    oc = hp
                    # psc0: [0:64] ctx hh0, row 64 den hh0 (v ones column)
                    # psc1: row 0 den hh1, [64:128] ctx hh1
                    pscs = [
                        psC.tile([65 + 63 * hh, 512], FP, tag=f"psc{hh}",
                                 bufs=1, name=f"psc{hh}")
                        for hh in range(2)
                    ]
                    ats = {}
                    pss_tiles = {}
                    for s4 in range(4):
                        t_lo, t_hi = TLO[s4], THI[s4]
                        S0 = P * s4
                        for hh in range(2):
                            po = HD * hh
                            bh = 4 * b + 2 * hp + hh
                            pss = psA.tile([P, 512], FP, tag="psA")
                            ksl = kT[po : po + HD, oc, tb + S0 : tb + S0 + P]
                            if t_lo:
                                nc.tensor.matmul(
                                    pss[:, 0:t_lo],
                                    lhsT=ksl,
                                    rhs=qTu[po : po + HD, oc, tb : tb + t_lo],
                                    start=True,
                                    stop=False,
                                )
                                nc.tensor.matmul(
                                    pss[:, 0:t_lo],
                                    lhsT=selmat[:, bh, :],
                                    rhs=arow_sb[:, 0:t_lo],
                                    start=False,
                                    stop=True,
                                )
                            nc.tensor.matmul(
                                pss[:, t_lo:T],
                                lhsT=ksl,
                                rhs=qTu[po : po + HD, oc, tb + t_lo : tb + T],
                                start=True,
                                stop=True,
                            )
                            at = attnp.tile([P, T], BF, tag="attn")
                            nc.scalar.activation(
                                out=at, in_=pss, func=EXP, scale=0.125
                            )
                            # both heads' multiplier strips live in one
                            # [128, 2, 288] PSUM tile per (pair, chunk),
                            # assembled by aligned PE transposes of the
                            # sheared window; col 0 is t = S0-128
                            o2 = o2s[bh]
                            if hh == 0:
                                psS = psP.tile(
                                    [P, 2, 288], BF, tag="psS", name="psS"
                                )
                                pss_tiles[s4] = psS
                            else:
                                psS = pss_tiles[s4]
                            nc.tensor.transpose(
                                psS[:, hh, 128:256],
                                o2[:, s4, 158:286],
                                ident_bf,
                            )
                            if s4 > 0:
                                nc.tensor.transpose(
                                    psS[:, hh, 0:128],
                                    o2[:, s4 - 1, 286:414],
                                    ident_bf,
                                )
                            if s4 < 3:
                                # rows 0:32 of o2's next block: t in
                                # [S0+128, S0+160)
                                nc.tensor.transpose(
                                    psS[:, hh, 256:288],
                                    o2[0:32, s4 + 1, 30:158],
                                    ident_bf[0:32, 0:32],
                                )
                            nc.vector.tensor_tensor(
                                at[:, t_lo:t_hi],
                                at[:, t_lo:t_hi],
                                psS[:, hh, t_lo - S0 + 128 : t_hi - S0 + 128],
                                MUL,
                            )
                            ats[(hh, s4)] = at
                    for s4 in range(4):
                        at0 = ats[(0, s4)]
                        nc.tensor.matmul(
                            pscs[0],
                            lhsT=v_sb[:, 4 * b + s4, 2 * hp, :],
                            rhs=at0,
                            start=(s4 == 0),
                            stop=(s4 == 3),
                        )
                        at1 = ats[(1, s4)]
                        nc.tensor.matmul(
                            pscs[1][HD:P, :],
                            lhsT=v_sb[:, 4 * b + s4, 2 * hp + 1, 0:HD],
                            rhs=at1,
                            start=(s4 == 0),
                            stop=(s4 == 3),
                        )
                        nc.tensor.matmul(
                            pscs[1][0:1, :],
                            lhsT=v_sb[:, 4 * b + s4, 2 * hp + 1, HD : HD + 1],
                            rhs=at1,
                            start=(s4 == 0),
                            stop=(s4 == 3),
                        )
                    if pending is not None:
                        emit_normalize(pending)
                    pending = (pscs, b, hp)
            emit_normalize(pending)
    nc.finalize()
    return nc


def _get_nc():
    if "nc" not in _CACHE:
        _CACHE["nc"] = _build_nc()
    return _CACHE["nc"]


def _make_in_maps(inputs):
    x = np.asarray(inputs["inputs"], dtype=np.float32)  # [16, 512, 256]
    enc = _enc_table()
    bf = ml_dtypes.bfloat16

    def wtile(w):
        # W [o, i] -> W.T [i, o] -> [p, (c o)] with i = c*128 + p
        return (
            np.asarray(w, np.float32)
            .T.reshape(2, P, D)
            .transpose(1, 0, 2)
            .reshape(P, 512)
        )

    def coltile(v):
        return np.asarray(v, np.float32).reshape(2, P).T  # [p, c]

    cbf = np.zeros((P, CBFW), np.float32)
    for off, w in [
        (WQ_OFF, inputs["Wq"]),
        (WK_OFF, inputs["Wk"]),
        (WV_OFF, inputs["Wv"]),
        (WO_OFF, inputs["Wo"]),
        (WP_OFF, inputs["Wpos"]),
    ]:
        cbf[:, off : off + 512] = wtile(w)
    encp = np.zeros((2, P, 64), np.float32)
    encp[:, :, 0:NR] = enc.T.reshape(2, P, NR)
    cbf[:, ENC_OFF : ENC_OFF + 128] = encp.transpose(1, 0, 2).reshape(P, 128)
    cbf[:, ID_OFF : ID_OFF + 128] = np.eye(P, dtype=np.float32)
    cbf[0, SEL_OFF : SEL_OFF + 64] = 1.0
    cbf[32, SEL_OFF + 64 : SEL_OFF + 128] = 1.0
    cbf[0, ROWS_OFF : ROWS_OFF + 256] = np.asarray(inputs["bv"], np.float32)
    cbf[32, ROWS_OFF : ROWS_OFF + 256] = np.asarray(inputs["bo"], np.float32)
    for bh in range(8):
        cbf[bh, SELM_OFF + 128 * bh : SELM_OFF + 128 * (bh + 1)] = 1.0

    vecs = np.zeros((P, 2, 8), np.float32)
    vecs[:, :, 0] = coltile(inputs["ln_gamma"])
    vecs[:, :, 1] = coltile(inputs["ln_beta"])
    vecs[:, :, 2] = coltile(inputs["bq"])
    vecs[:, :, 3] = coltile(inputs["bk"])
    vecs[:, :, 4] = coltile(inputs["bpos"])
    vecs[:, :, 5] = coltile(np.asarray(inputs["u_bias"], np.float32).reshape(D))
    vecs[:, :, 6] = coltile(np.asarray(inputs["v_bias"], np.float32).reshape(D))

    common = {
        "cbf": np.ascontiguousarray(cbf.astype(bf)),
        "cfp": np.ascontiguousarray(vecs.reshape(P, 16)),
    }
    in_maps = []
    for core in range(NCORES):
        m = dict(common)
        m["x"] = np.ascontiguousarray(
            x[BPC * core : BPC * (core + 1)].reshape(N, D)
        )
        in_maps.append(m)
    return in_maps


def run(inputs, trace=False):
    nc = _get_nc()
    in_maps = _make_in_maps(inputs)
    res = run_bass_kernel_spmd(
        nc, in_maps, core_ids=list(range(NCORES)), trace=trace
    )
    outs = [np.asarray(r["out"]) for r in res.results]
    full = np.concatenate(outs, axis=0).reshape(B, T, D).astype(np.float32)
    return full, res


def kernel(**inputs) -> np.ndarray:
    full, _ = run(inputs, trace=False)
    return full
